# revision 1
# baseline (speedup 1.0000x reference)
"""Sparse (segment + causal) GQA attention on 8 Trainium2 NeuronCores.

Problem: nn_AttentionOp_27719718928719
  query (2, 1024, 32, 128) f32, key/value (2, 1024, 8, 128) f32,
  decoder_segment_ids (2, 1024) i32 (sorted) -> out (2, 1024, 32, 128) f32

Sharding: core c owns kv-head c and its 4 GQA query heads, both batches.
Perfect shard (no replication): Q, K, V, and the output all split 8 ways, and
the compiled program is identical on every core (the block schedule depends
only on the segment ids, which all cores share).

Device algorithm, one unit per (batch b, 128-query block tj) with all 4 heads
fused along the free axis (512 wide everywhere):
  for each valid key block si (causal + segment overlap, host-computed):
    S^T[s, (h,t)] = K[si]^T Q      3 bf16 hi/lo matmuls -> fp32-grade logits
    S^T += bias(s) * 1(h,t)        rank-1 matmuls adding -60000 to keys
                                   outside a t-span's segment (masking on PE)
    P^T = exp(S^T)                 ACT, writes float32r directly to SBUF
    causal zero (diag blocks only) one Pool affine_select for all 4 heads
    outT[d, (h,t)] += V[si]' P^T   f32r matmul, PSUM-accumulated over si
    sums[1, (h,t)] += 1' P^T       f32r ones matmul
  stage PSUM -> SBUF (DVE), DMA out.
No softmax max-subtraction: logits are O(+-50) so exp stays in fp32 range and
exp(x)/sum(exp(x)) matches the reference's exp(x-max)/sum(exp(x-max)) exactly.
Host does the (cheap) normalization out/sums and all layout transposes.
"""

import numpy as np
import ml_dtypes

B, T, S, NQ, NKV, D = 2, 1024, 1024, 32, 8, 128
G = NQ // NKV
BLK = 128
NBLK = S // BLK  # 8
W = G * BLK  # 512: fused 4-head free width
N_CORES = 8
HLOC = NQ // N_CORES  # 4
MASK_BIAS = -60000.0

_compiled_cache = {}

# Test-only knobs (the grading path never sets these): when TRACE is true the
# SPMD run captures an NTFF profile into TRACE_DIR.
TRACE = False
TRACE_DIR = None

# QK product mode: "hilo" = 3 bf16 hi/lo matmuls (fp32-grade logits),
# "f32r" = single fp32r matmul (3x less PE time, ~30x more logit error).
QK_MODE = "hilo"


def _split_bf16(x):
    hi = x.astype(ml_dtypes.bfloat16)
    lo = (x - hi.astype(np.float32)).astype(ml_dtypes.bfloat16)
    return hi, lo


def _segment_structure(seg):
    """Block schedule for one batch's (sorted) segment ids.

    Returns (sched, bias_classes):
      sched[tj] = list of (si, bias_ops, diag) where bias_ops is a list of
        (a, b, cls) adding bias class `cls` to t-columns [a, b) of the block,
        and diag marks the causal in-block mask.
      bias_classes = list of np bool arrays [BLK]: True where the key row gets
        MASK_BIAS.
    """
    seg = np.asarray(seg)
    t_idx = np.arange(S)
    seg_start = np.zeros(S, np.int64)
    seg_end = np.zeros(S, np.int64)
    for v in np.unique(seg):
        m = seg == v
        lo, hi = np.argmax(m), S - np.argmax(m[::-1])
        seg_start[m], seg_end[m] = lo, hi
    valid_ts = (t_idx[None, :] <= t_idx[:, None]) & (seg[None, :] == seg[:, None])
    v4 = valid_ts.reshape(NBLK, BLK, NBLK, BLK)
    vblk = v4.any(axis=(1, 3))  # [tj, si]
    fblk = v4.all(axis=(1, 3))

    classes = []  # list of np.bool arrays
    cls_key = {}

    def class_id(mask_rows):
        key = mask_rows.tobytes()
        if key not in cls_key:
            cls_key[key] = len(classes)
            classes.append(mask_rows.copy())
        return cls_key[key]

    sched = []
    for tj in range(NBLK):
        entries = []
        sis = [si for si in range(NBLK) if vblk[tj, si]]
        assert sis == list(range(min(sis), max(sis) + 1))
        for si in sis:
            bias_ops = []
            if not fblk[tj, si] and not (si == tj and _only_causal(v4, tj, si)):
                tcols = np.arange(tj * BLK, (tj + 1) * BLK)
                lo_rel = np.clip(seg_start[tcols] - si * BLK, 0, BLK)
                hi_rel = np.clip(seg_end[tcols] - si * BLK, 0, BLK)
                a = 0
                for i in range(1, BLK + 1):
                    if i == BLK or lo_rel[i] != lo_rel[a] or hi_rel[i] != hi_rel[a]:
                        lo, hi = int(lo_rel[a]), int(hi_rel[a])
                        rows = np.ones(BLK, dtype=bool)
                        rows[lo:hi] = False  # False -> keep
                        if rows.any():
                            bias_ops.append((a, i, class_id(rows)))
                        a = i
            entries.append((si, bias_ops, si == tj))
        sched.append(entries)
    return sched, classes


def _only_causal(v4, tj, si):
    """True if block (tj, si)'s invalid entries are exactly the causal ones."""
    blk = v4[tj, :, si, :]  # [t, s]
    t = np.arange(BLK)[:, None] + tj * BLK
    s = np.arange(BLK)[None, :] + si * BLK
    return bool((blk == (s <= t)).all())


def _build_program(scheds, all_classes, qk_mode):
    """Build the SPMD Bass program. scheds/all_classes indexed by batch."""
    import concourse.bass as bass  # noqa: F401
    from concourse import bacc
    import concourse.mybir as mybir
    import concourse.tile as tile

    DT = mybir.dt
    F32R = DT.float32r
    QDT = DT.bfloat16 if qk_mode == "hilo" else F32R
    ncls = [len(c) for c in all_classes]
    nc = bacc.Bacc(None, target_bir_lowering=False, debug=False)

    qhi_d = nc.dram_tensor("qhi", [B, D, NBLK, HLOC, BLK], QDT, kind="ExternalInput").ap()
    khi_d = nc.dram_tensor("khi", [B, D, S], QDT, kind="ExternalInput").ap()
    if qk_mode == "hilo":
        qlo_d = nc.dram_tensor("qlo", [B, D, NBLK, HLOC, BLK], QDT, kind="ExternalInput").ap()
        klo_d = nc.dram_tensor("klo", [B, D, S], QDT, kind="ExternalInput").ap()
    v_d = nc.dram_tensor("v", [B, NBLK, BLK, D], DT.float32, kind="ExternalInput").ap()
    ones_d = nc.dram_tensor("ones_in", [BLK, 1], DT.float32, kind="ExternalInput").ap()
    nbias = max(1, sum(ncls))
    bias_d = nc.dram_tensor("bias_in", [1, nbias * BLK], DT.bfloat16, kind="ExternalInput").ap()
    outT_d = nc.dram_tensor("outT", [B, NBLK, D, W], DT.float32, kind="ExternalOutput").ap()
    sums_d = nc.dram_tensor("sums", [1, B * NBLK * W], DT.float32, kind="ExternalOutput").ap()

    cls_base = [0, ncls[0]]  # class index offset per batch

    with tile.TileContext(nc) as tc:
        with (
            tc.tile_pool(name="const", bufs=1) as constp,
            tc.tile_pool(name="qkv", bufs=1) as qkv,
            tc.tile_pool(name="pt", bufs=6) as ptp,
            tc.tile_pool(name="stage", bufs=4) as stage,
            tc.tile_pool(name="sumstage", bufs=1) as sumstage,
            tc.tile_pool(name="ps_s", bufs=3, space="PSUM") as ps_s,
            tc.tile_pool(name="ps_o", bufs=3, space="PSUM") as ps_o,
            tc.tile_pool(name="ps_m", bufs=2, space="PSUM") as ps_m,
        ):
            # b=0 inputs first so compute can start while b=1 still loads
            k_hi = qkv.tile([D, B, S], QDT)
            v_t = qkv.tile([BLK, B, NBLK, D], F32R)
            q_hi = qkv.tile([D, B, NBLK, HLOC, BLK], QDT)
            if qk_mode == "hilo":
                k_lo = qkv.tile([D, B, S], QDT)
                q_lo = qkv.tile([D, B, NBLK, HLOC, BLK], QDT)
            ones_t = constp.tile([BLK, 1], F32R)
            ones_bf = constp.tile([1, HLOC, BLK], DT.bfloat16)
            bias_t = constp.tile([1, nbias * BLK], DT.bfloat16)
            exp_bias = constp.tile([BLK, 1], mybir.dt.float32)
            nc.vector.memset(exp_bias, -30.0)
            s_all = sumstage.tile([1, B * NBLK * W], mybir.dt.float32)

            # All loads on the Sync engine, ordered so the first compute unit's
            # inputs (b=0, tj=0) land within a few microseconds; later chunks
            # overlap compute. Chunking also spreads transfers across queues.
            def load_q(b, lo_blk, hi_blk):
                sl = np.s_[lo_blk:hi_blk]
                if qk_mode == "hilo":
                    nc.sync.dma_start(out=q_hi[:, b, sl], in_=qhi_d[b, :, sl])
                    nc.sync.dma_start(out=q_lo[:, b, sl], in_=qlo_d[b, :, sl])
                else:
                    nc.sync.dma_start(
                        out=q_hi[:, b, sl], in_=qhi_d[b, :, sl].bitcast(F32R)
                    )

            def load_kv(b):
                if qk_mode == "hilo":
                    nc.sync.dma_start(out=k_hi[:, b], in_=khi_d[b])
                    nc.sync.dma_start(out=k_lo[:, b], in_=klo_d[b])
                else:
                    nc.sync.dma_start(out=k_hi[:, b], in_=khi_d[b].bitcast(F32R))
                nc.sync.dma_start(
                    out=v_t[:, b], in_=v_d[b].bitcast(F32R).rearrange("si p d -> p si d")
                )

            # K/V for b=0 on the Pool engine: its descriptor generation runs in
            # parallel with Sync's q chunks, and Pool's first affine_select
            # isn't needed until well after these land.
            if qk_mode == "hilo":
                nc.gpsimd.dma_start(out=k_hi[:, 0], in_=khi_d[0])
            else:
                nc.gpsimd.dma_start(out=k_hi[:, 0], in_=khi_d[0].bitcast(F32R))
            load_q(0, 0, 1)
            if qk_mode == "hilo":
                nc.gpsimd.dma_start(out=k_lo[:, 0], in_=klo_d[0])
            nc.gpsimd.dma_start(
                out=v_t[:, 0, 0:4],
                in_=v_d[0, 0:4].bitcast(F32R).rearrange("si p d -> p si d"),
            )
            nc.sync.dma_start(out=ones_t, in_=ones_d.bitcast(F32R))
            nc.sync.dma_start(out=bias_t, in_=bias_d)
            nc.vector.memset(ones_bf, 1.0)
            load_q(0, 1, 2)
            load_q(0, 2, 4)
            nc.sync.dma_start(
                out=v_t[:, 0, 4:],
                in_=v_d[0, 4:].bitcast(F32R).rearrange("si p d -> p si d"),
            )
            load_q(0, 4, NBLK)
            load_kv(1)
            load_q(1, 0, 4)
            load_q(1, 4, NBLK)

            for b in range(B):
                # b=1 runs its lightest unit (tj0: diag only) last so the
                # final pipeline drain is as short as possible; b=0 keeps
                # ascending order to match the input DMA arrival order.
                tj_order = list(range(NBLK)) if b == 0 else list(range(1, NBLK)) + [0]
                for tj in tj_order:
                    entries = scheds[b][tj]
                    outp = ps_o.tile([D, W], mybir.dt.float32)
                    sm = ps_m.tile([1, W], mybir.dt.float32)
                    n_e = len(entries)
                    pts = []
                    for idx, (si, bias_ops, diag) in enumerate(entries):
                        st = ps_s.tile([BLK, HLOC, BLK], mybir.dt.float32)
                        kh = k_hi[:, b, si * BLK:(si + 1) * BLK]
                        qh = q_hi[:, b, tj]
                        last_qk = len(bias_ops) == 0
                        if qk_mode == "hilo":
                            kl = k_lo[:, b, si * BLK:(si + 1) * BLK]
                            ql = q_lo[:, b, tj]
                            nc.tensor.matmul(st, kh, qh, start=True, stop=False,
                                             skip_group_check=True)
                            nc.tensor.matmul(st, kh, ql, start=False, stop=False,
                                             skip_group_check=True)
                            nc.tensor.matmul(st, kl, qh, start=False, stop=last_qk,
                                             skip_group_check=True)
                        else:
                            nc.tensor.matmul(st, kh, qh, start=True, stop=last_qk,
                                             skip_group_check=True)
                        for bi, (a, e, cls) in enumerate(bias_ops):
                            cid = cls_base[b] + cls
                            nc.tensor.matmul(
                                st[:, :, a:e],
                                bias_t[:, cid * BLK:(cid + 1) * BLK],
                                ones_bf[:, :, :e - a],
                                start=False, stop=bi == len(bias_ops) - 1,
                                skip_group_check=True,
                            )

                        # exp(x - 30): headroom against fp32 exp overflow for
                        # unlucky logit maxima; cancels in out/sums exactly.
                        pt = ptp.tile([BLK, HLOC, BLK], F32R)
                        nc.scalar.activation(
                            out=pt, in_=st, func=mybir.ActivationFunctionType.Exp,
                            bias=exp_bias,
                        )
                        if diag:
                            # keep s <= t for every head: iota = -4x + h + 4y,
                            # >= 0 iff y >= x (h in 0..3 can't flip it)
                            nc.gpsimd.affine_select(
                                out=pt, in_=pt, compare_op=mybir.AluOpType.is_ge,
                                fill=0.0, base=0,
                                pattern=[[1, HLOC], [HLOC, BLK]],
                                channel_multiplier=-HLOC,
                            )

                        first, last = idx == 0, idx == n_e - 1
                        nc.tensor.matmul(outp, v_t[:, b, si], pt,
                                         start=first, stop=last,
                                         skip_group_check=True)
                        pts.append(pt)

                    # sums back-to-back after the si loop: the ones weights
                    # load once per unit instead of once per si-step
                    for j, ptt in enumerate(pts):
                        nc.tensor.matmul(sm, ones_t, ptt,
                                         start=j == 0, stop=j == len(pts) - 1,
                                         skip_group_check=True)

                    o_sb = stage.tile([D, W], mybir.dt.float32)
                    nc.vector.tensor_copy(out=o_sb, in_=outp)
                    nc.vector.tensor_copy(
                        out=s_all[:, (b * NBLK + tj) * W:(b * NBLK + tj + 1) * W],
                        in_=sm,
                    )
                    nc.sync.dma_start(out=outT_d[b, tj], in_=o_sb)
                nc.sync.dma_start(
                    out=sums_d[:, b * NBLK * W:(b + 1) * NBLK * W],
                    in_=s_all[:, b * NBLK * W:(b + 1) * NBLK * W],
                )
    nc.compile()
    return nc


def kernel(query, key, value, decoder_segment_ids):
    from concourse.bass_utils import run_bass_kernel_spmd

    query = np.asarray(query, dtype=np.float32)
    key = np.asarray(key, dtype=np.float32)
    value = np.asarray(value, dtype=np.float32)
    seg = np.asarray(decoder_segment_ids, dtype=np.int32)

    structs = [_segment_structure(seg[b]) for b in range(B)]
    scheds = [s[0] for s in structs]
    all_classes = [s[1] for s in structs]
    sig = tuple(
        tuple(tuple((si, tuple(ops), diag) for (si, ops, diag) in entries)
              for entries in sched)
        for sched in scheds
    ) + tuple(c.tobytes() for cl in all_classes for c in cl) + (QK_MODE,)
    nc = _compiled_cache.get(sig)
    if nc is None:
        nc = _build_program(scheds, all_classes, QK_MODE)
        _compiled_cache[sig] = nc

    ones_in = np.ones((BLK, 1), dtype=np.float32)
    nbias = max(1, sum(len(c) for c in all_classes))
    bias_in = np.zeros((1, nbias * BLK), dtype=ml_dtypes.bfloat16)
    i = 0
    for cl in all_classes:
        for rows in cl:
            bias_in[0, i * BLK:(i + 1) * BLK] = np.where(rows, MASK_BIAS, 0.0)
            i += 1

    in_maps = []
    for c in range(N_CORES):
        q_c = query[:, :, c * HLOC:(c + 1) * HLOC, :]  # (B, T, HLOC, D)
        # -> (B, D, NBLK, HLOC, BLK): element [b,d,tj,h,y] = q_c[b, tj*128+y, h, d]
        qT = np.ascontiguousarray(
            q_c.transpose(0, 3, 1, 2)  # (B, D, T, HLOC)
            .reshape(B, D, NBLK, BLK, HLOC)
            .transpose(0, 1, 2, 4, 3)
        )
        kT = np.ascontiguousarray(key[:, :, c, :].transpose(0, 2, 1))  # (B, D, S)
        v_c = np.ascontiguousarray(value[:, :, c, :].reshape(B, NBLK, BLK, D))
        m = {"v": v_c, "ones_in": ones_in, "bias_in": bias_in}
        if QK_MODE == "hilo":
            m["qhi"], m["qlo"] = _split_bf16(qT)
            m["khi"], m["klo"] = _split_bf16(kT)
        else:
            m["qhi"], m["khi"] = qT, kT
        in_maps.append(m)

    kwargs = {}
    if TRACE:
        kwargs = dict(trace=True, tmpdir=TRACE_DIR)
    res = run_bass_kernel_spmd(nc, in_maps, core_ids=list(range(N_CORES)), **kwargs)
    kernel.last_results = res

    out = np.empty((B, T, NQ, D), dtype=np.float32)
    for c in range(N_CORES):
        outT = res.results[c]["outT"]  # (B, NBLK, D, W) with W = (HLOC, BLK)
        sums = res.results[c]["sums"]  # (1, B*NBLK*W)
        o = outT.reshape(B, NBLK, D, HLOC, BLK)
        s = sums.reshape(B, NBLK, HLOC, BLK)
        # out[b, tj*128+y, c*4+h, d] = o[b, tj, d, h, y] / s[b, tj, h, y]
        o = o.transpose(0, 1, 4, 3, 2).reshape(B, T, HLOC, D)
        s = s.transpose(0, 1, 3, 2).reshape(B, T, HLOC)
        out[:, :, c * HLOC:(c + 1) * HLOC, :] = o / s[:, :, :, None]
    return out

